# revision 2
# baseline (speedup 1.0000x reference)
"""Transformer-XL relative multi-head attention, 8-way sharded on Trainium2.

Self-contained harness entry: kernel(**inputs) -> np.ndarray [4, 1024, 1024].

Sharding: core c handles batch b = c//2 and head-half hh = c%2 (8 of 16
heads). Each core computes a partial output (its heads' contribution
through Wo); the host unshard sums the two partials per batch (row-parallel
tensor parallelism for the output projection).
"""

import os
import sys

sys.path.insert(0, "/opt/trn_rl_repo")

import numpy as np


import concourse.bass as bass
import concourse.mybir as mybir
from concourse.tile import TileContext, ScopedClock

F32 = mybir.dt.float32
F32R = mybir.dt.float32r
F16 = mybir.dt.float16
AF = mybir.ActivationFunctionType
OP = mybir.AluOpType

S, T, D, HC, DK, P = 1024, 2048, 1024, 8, 64, 128
DH = HC * DK  # 512, head-slice width per core
NQT = S // P  # 8 query tiles
WMAX = 2048 + 127 + 128  # padded shear slot width (>= max W)
SLOT = P * WMAX + P  # dram scratch slot elements (pad for strided read)
EXP_BIAS = -7.0
NEG_BIG = -60000.0


def _patched_drain_and_barrier(self, tick_clock, wait_clock):
    # The walrus build in this container caps sync-waits per instruction;
    # Tile's stock tail drain carries one wait per live proc. Emit one SP nop
    # per wait instead, then the drain.
    dummy = mybir.InstNoOp(name="drain-wait-probe", ins=[], outs=[])
    dummy.engine = mybir.EngineType.SP
    wait_clock.add_sem_waits(dummy, ScopedClock({None: tick_clock.global_clock}))
    waits = []
    if dummy.sync_info is not None and dummy.sync_info.on_wait:
        waits = [(w.ant_name, w.wait_value) for w in dummy.sync_info.on_wait]
    assert self.sems is not None
    name2sem = {h.name: h for h in self.sems.allocated().values()}
    for name, val in waits:
        self.nc.sync.nop().wait_op(name2sem[name], val, "sem-ge")
    self.nc.sync.drain()
    self.nc.all_engine_barrier()
    popped = self.nc._tile_sem_poison_stack.pop()
    assert popped is self._sem_poison
    self.nc.clear_and_free_semaphores(list(self.sems.allocated().values()))
    self.nc.all_engine_barrier()


TileContext._drain_and_barrier = _patched_drain_and_barrier



def _split_multi_waits(nc, max_waits=1):
    """Walrus in this container rejects instructions carrying more than a
    couple of sync waits. Hoist extras onto same-engine NoOps just before
    the instruction (sequential on the engine, so semantics unchanged)."""
    for f in nc.m.functions:
        for bb in f.blocks:
            out = []
            changed = False
            for inst in bb.instructions:
                si = inst.sync_info
                if si is not None and si.on_wait and len(si.on_wait) > max_waits:
                    waits = list(si.on_wait)
                    for j, w in enumerate(waits[:-max_waits]):
                        nop = mybir.InstNoOp(
                            name=f"{inst.name}-wsplit{j}", ins=[], outs=[])
                        nop.engine = inst.engine
                        nop.sync_info = mybir.SyncInfo(on_wait=[w], on_update=[])
                        out.append(nop)
                    inst.sync_info = mybir.SyncInfo(
                        on_wait=waits[-max_waits:],
                        on_update=list(si.on_update))
                    changed = True
                out.append(inst)
            if changed:
                bb.instructions = out


def kq_of(qi):  # valid key count for query tile qi (keys j <= i + 1024)
    return (qi + 9) * P


def build_nc(split_waits=True):
    nc = bass.Bass(target_bir_lowering=True)

    # fp32 inputs are declared float32r: same bits, PE runs the fp22
    # full-rate path on them.
    qT = nc.declare_dram_parameter("qT", [D, S], F32R, isOutput=False)
    kT = nc.declare_dram_parameter("kT", [D, T], F32R, isOutput=False)
    vT = nc.declare_dram_parameter("vT", [D, T], F32R, isOutput=False)
    RT = nc.declare_dram_parameter("RT", [D, T], F32R, isOutput=False)
    Wq = nc.declare_dram_parameter("Wq", [D, DH], F32R, isOutput=False)
    Wk = nc.declare_dram_parameter("Wk", [D, DH], F32R, isOutput=False)
    Wv = nc.declare_dram_parameter("Wv", [D, DH], F32R, isOutput=False)
    Wr = nc.declare_dram_parameter("Wr", [D, DH], F32R, isOutput=False)
    Wo16 = nc.declare_dram_parameter("Wo16", [DH, D], F16, isOutput=False)
    ub = nc.declare_dram_parameter("ub", [P, 4], F32, isOutput=False)
    vb = nc.declare_dram_parameter("vb", [P, 4], F32, isOutput=False)
    atril = nc.declare_dram_parameter("atril", [P, P], mybir.dt.uint8, isOutput=False)
    outp = nc.declare_dram_parameter("out", [S, D], F32, isOutput=True)

    with TileContext(nc) as tc:
        with (
            tc.tile_pool(name="persist", bufs=1) as pp,
            tc.tile_pool(name="consts", bufs=1) as cp,
        ):
            # persistent fp16 tensors (partition = dk within head-pair tile)
            quT = pp.tile([P, 4 * S], F16)      # (qh+u).T   blocks hp
            qvT = pp.tile([P, 4 * S], F16)      # (qh+v).T
            khT = pp.tile([P, 4 * T], F16)
            rh2T = pp.tile([P, 4 * 3072], F16)
            vh16 = pp.tile([P, 16 * (HC * 65)], F16)  # per key tile: 8 heads x (64+1)
            concatT = pp.tile([P, 4 * S], F16)
            WoS = pp.tile([P, 4 * D], F16)

            ub_sb = cp.tile([P, 4], F32)
            vb_sb = cp.tile([P, 4], F32)
            atril_sb = cp.tile([P, P], mybir.dt.uint8)
            negbig_sb = cp.tile([P, P], F32)
            expb_sb = cp.tile([P, 1], F32)
            nc.vector.memset(expb_sb[:], EXP_BIAS)

            nc.sync.dma_start(out=ub_sb[:], in_=ub[:])
            nc.sync.dma_start(out=vb_sb[:], in_=vb[:])
            nc.sync.dma_start(out=atril_sb[:], in_=atril[:])
            nc.vector.memset(negbig_sb[:], NEG_BIG)
            # WoS layout [128, dt*1024 + o] <- Wo16[(dt p), o]
            for dt_ in range(4):
                nc.sync.dma_start(
                    out=WoS[:, dt_ * D : (dt_ + 1) * D],
                    in_=Wo16[dt_ * P : (dt_ + 1) * P, :],
                )

            # ---------------- projections ----------------
            def load_w(pool, wparam):
                wsb = pool.tile([P, 8 * DH], F32R, tag="wsb")
                for kd in range(8):
                    nc.sync.dma_start(
                        out=wsb[:, kd * DH : (kd + 1) * DH],
                        in_=wparam[kd * P : (kd + 1) * P, :],
                    )
                return wsb

            # qhT-style projection: out[512, ncols] = W_s @ xT, evacuated by fn
            def proj_T(pool, psum, wsb, xparam, ncols, evac):
                nth = ncols // 1024
                for th in range(nth):
                    psums = {k: psum.tile([P, 512], F32, tag="proj", name="projps")
                             for k in [(d, t2) for d in range(4) for t2 in range(2)]}
                    for kd in range(8):
                        xsb = pool.tile([P, 1024], F32R, tag="xstage")
                        nc.sync.dma_start(
                            out=xsb[:],
                            in_=xparam[kd * P : (kd + 1) * P,
                                       th * 1024 : (th + 1) * 1024],
                        )
                        for dot in range(4):
                            for tc2 in range(2):
                                nc.tensor.matmul(
                                    psums[(dot, tc2)][:],
                                    wsb[:, kd * DH + dot * P : kd * DH + (dot + 1) * P],
                                    xsb[:, tc2 * 512 : (tc2 + 1) * 512],
                                    start=(kd == 0),
                                    stop=(kd == 7),
                                )
                    for dot in range(4):
                        for tc2 in range(2):
                            evac(psums[(dot, tc2)], dot, th * 1024 + tc2 * 512)

            with (
                tc.tile_pool(name="projp", bufs=3) as jp,
                tc.tile_pool(name="projw", bufs=2) as jw,
                tc.tile_pool(name="rhtmp", bufs=1) as jr,
                tc.tile_pool(name="projpsum", bufs=8, space="PSUM") as jps,
            ):
                wsb = load_w(jw, Wq)

                def evac_q(ps, dot, col):
                    nc.vector.tensor_scalar(
                        quT[:, dot * S + col : dot * S + col + 512], ps[:],
                        ub_sb[:, dot : dot + 1], None, OP.add)
                    nc.vector.tensor_scalar(
                        qvT[:, dot * S + col : dot * S + col + 512], ps[:],
                        vb_sb[:, dot : dot + 1], None, OP.add)

                proj_T(jp, jps, wsb, qT, S, evac_q)

                wsb = load_w(jw, Wk)

                def evac_k(ps, dot, col):
                    nc.vector.tensor_copy(
                        khT[:, dot * T + col : dot * T + col + 512], ps[:])

                proj_T(jp, jps, wsb, kT, T, evac_k)

                rhT = jr.tile([P, 4 * T], F16, tag="rhT")
                wsb = load_w(jw, Wr)

                def evac_r(ps, dot, col):
                    nc.vector.tensor_copy(
                        rhT[:, dot * T + col : dot * T + col + 512], ps[:])

                proj_T(jp, jps, wsb, RT, T, evac_r)

                # rh2T[:, m'] = rhT[:, (m' + 1023) % 2048], m' in [0, 3072)
                for dot in range(4):
                    nc.vector.tensor_copy(
                        rh2T[:, dot * 3072 : dot * 3072 + 1025],
                        rhT[:, dot * T + 1023 : dot * T + 2048])
                    nc.vector.tensor_copy(
                        rh2T[:, dot * 3072 + 1025 : dot * 3072 + 3072],
                        rhT[:, dot * T : dot * T + 2047])

                # vh (untransposed): per key tile tt, psum [128 keys, 512 dh]
                wsb = load_w(jw, Wv)
                for tg in range(2):
                    vps = {tl: jps.tile([P, 512], F32, tag="proj", name="vhps")
                           for tl in range(8)}
                    for kd in range(8):
                        vsb = jp.tile([P, 1024], F32R, tag="xstage")
                        nc.sync.dma_start(
                            out=vsb[:],
                            in_=vT[kd * P : (kd + 1) * P,
                                   tg * 1024 : (tg + 1) * 1024],
                        )
                        for tl in range(8):
                            nc.tensor.matmul(
                                vps[tl][:],
                                vsb[:, tl * P : (tl + 1) * P],
                                wsb[:, kd * DH : (kd + 1) * DH],
                                start=(kd == 0),
                                stop=(kd == 7),
                            )
                    for tl in range(8):
                        tt = tg * 8 + tl
                        ps = vps[tl]
                        base = tt * (HC * 65)
                        dst = bass.AP(vh16.tensor, vh16.offset + base,
                                      [[vh16.tensor.shape[1], P], [65, HC], [1, DK]])
                        nc.vector.tensor_copy(dst, ps[:].rearrange("p (h c) -> p h c", h=HC))
                        ones = bass.AP(vh16.tensor, vh16.offset + base + DK,
                                       [[vh16.tensor.shape[1], P], [65, HC]])
                        nc.vector.memset(ones, 1.0)

            # ---------------- attention ----------------
            with (
                tc.tile_pool(name="att_m", bufs=2) as mp,
                tc.tile_pool(name="att_sc", bufs=2) as scp,
                tc.tile_pool(name="att_att", bufs=4) as atp,
                tc.tile_pool(name="att_tr", bufs=3) as trp,
                tc.tile_pool(name="dram", bufs=4, space="DRAM") as dp,
                tc.tile_pool(name="ps_m", bufs=2, space="PSUM") as psm,
                tc.tile_pool(name="ps_ac", bufs=4, space="PSUM") as psac,
                tc.tile_pool(name="ps_o", bufs=2, space="PSUM") as pso,
                tc.tile_pool(name="smalls", bufs=4) as smp,
            ):
                for hp in range(4):
                    att_tiles = {}
                    for qi in range(NQT):
                        KQ = kq_of(qi)
                        W = KQ + 127
                        for h in range(2):
                            pr = slice(h * DK, (h + 1) * DK)
                            # position-score matrix M [128, W]
                            msb = mp.tile([P, WMAX], F16, tag="msb")
                            nwc = (W + 511) // 512
                            for wc in range(nwc):
                                nw = min(512, W - wc * 512)
                                mps = psm.tile([P, 512], F32, tag="mps")
                                nc.tensor.matmul(
                                    mps[:, :nw],
                                    qvT[pr, hp * S + qi * P : hp * S + (qi + 1) * P],
                                    rh2T[pr, hp * 3072 + qi * P + wc * 512 :
                                         hp * 3072 + qi * P + wc * 512 + nw],
                                    start=True, stop=True,
                                )
                                nc.vector.tensor_copy(
                                    msb[:, wc * 512 : wc * 512 + nw], mps[:, :nw])
                            # shear via HBM: write rows stride W, read stride W+1
                            mdr = dp.tile([SLOT], F16, tag="mscr")
                            nc.sync.dma_start(
                                out=bass.AP(mdr.tensor, mdr.offset, [[W, P], [1, W]]),
                                in_=msb[:, :W],
                            )
                            bd = mp.tile([P, T], F16, tag="bd")
                            nc.sync.dma_start(
                                out=bd[:, :KQ],
                                in_=bass.AP(mdr.tensor, mdr.offset,
                                            [[W + 1, P], [1, KQ]]),
                            )
                            # content scores + combine + exp
                            att = atp.tile([P, T], F16, tag="att")
                            att_tiles[(h, qi)] = att
                            ssb = scp.tile([P, T], F32, tag="ssb")
                            nkc = (KQ + 511) // 512
                            for kc in range(nkc):
                                nk = min(512, KQ - kc * 512)
                                acps = psac.tile([P, 512], F32, tag="acps")
                                nc.tensor.matmul(
                                    acps[:, :nk],
                                    quT[pr, hp * S + qi * P : hp * S + (qi + 1) * P],
                                    khT[pr, hp * T + kc * 512 : hp * T + kc * 512 + nk],
                                    start=True, stop=True,
                                )
                                nc.vector.tensor_tensor(
                                    ssb[:, kc * 512 : kc * 512 + nk],
                                    acps[:, :nk],
                                    bd[:, kc * 512 : kc * 512 + nk],
                                    OP.add,
                                )
                            # causal boundary: block kj = qi+8 is tril
                            nc.vector.copy_predicated(
                                ssb[:, KQ - P : KQ], atril_sb[:], negbig_sb[:])
                            sums = smp.tile([P, 1], F32, tag="sums")
                            nc.scalar.activation(
                                att[:, :KQ], ssb[:, :KQ], AF.Exp,
                                bias=expb_sb[:], scale=0.125,
                                accum_out=sums[:])
                            recip_q = smp.tile([P, 1], F32, tag="recipq")
                            nc.vector.reciprocal(recip_q[:], sums[:])
                            nc.vector.tensor_scalar(
                                att[:, :KQ], att[:, :KQ], recip_q[:], None,
                                OP.mult)

                        if qi % 2 == 1:
                            # AV for query tiles (qi-1, qi), all valid keys
                            KQ0, KQ1 = kq_of(qi - 1), kq_of(qi)
                            njt = KQ1 // P
                            for h in range(2):
                                a0 = att_tiles.pop((h, qi - 1))
                                a1 = att_tiles.pop((h, qi))
                                nc.vector.memset(a0[:, KQ0:KQ1], 0.0)
                                ops = pso.tile([P, 256], F32, tag="ops")
                                for jt in range(njt):
                                    atr = trp.tile([P, 256], F16, tag="atr")
                                    nc.sync.dma_start_transpose(
                                        out=atr[:, 0:P],
                                        in_=a0[:, jt * P : (jt + 1) * P])
                                    nc.sync.dma_start_transpose(
                                        out=atr[:, P : 2 * P],
                                        in_=a1[:, jt * P : (jt + 1) * P])
                                    nc.tensor.matmul(
                                        ops[:65, :],
                                        vh16[:, jt * (HC * 65) + (hp * 2 + h) * 65 :
                                             jt * (HC * 65) + (hp * 2 + h) * 65 + 65],
                                        atr[:],
                                        start=(jt == 0), stop=(jt == njt - 1),
                                    )
                                qa = (qi - 1) // 2
                                nc.vector.tensor_copy(
                                    concatT[h * DK : (h + 1) * DK,
                                            hp * S + qa * 256 : hp * S + (qa + 1) * 256],
                                    ops[0:DK, :])

            # ---------------- output projection ----------------
            with (
                tc.tile_pool(name="outp", bufs=3) as op_,
                tc.tile_pool(name="outpsum", bufs=4, space="PSUM") as ops_,
            ):
                for it in range(8):
                    for oc in range(2):
                        ps = ops_.tile([P, 512], F32, tag="out")
                        for dt in range(4):
                            nc.tensor.matmul(
                                ps[:],
                                concatT[:, dt * S + it * P : dt * S + (it + 1) * P],
                                WoS[:, dt * D + oc * 512 : dt * D + (oc + 1) * 512],
                                start=(dt == 0), stop=(dt == 3),
                            )
                        osb = op_.tile([P, 512], F32, tag="osb")
                        nc.vector.tensor_copy(osb[:], ps[:])
                        nc.sync.dma_start(
                            out=outp[it * P : (it + 1) * P, oc * 512 : (oc + 1) * 512],
                            in_=osb[:])

    if split_waits:
        _split_multi_waits(nc)
    return nc


def prep_core_inputs(core, q, k, v, u, v_bias, Wq, Wk, Wv, Wr, Wo, R):
    b, hh = core // 2, core % 2
    sl = slice(hh * DH, (hh + 1) * DH)
    c = np.ascontiguousarray
    return {
        "qT": c(q[b].T),
        "kT": c(k[b].T),
        "vT": c(v[b].T),
        "RT": c(R.T),
        "Wq": c(Wq[sl, :].T),
        "Wk": c(Wk[sl, :].T),
        "Wv": c(Wv[sl, :].T),
        "Wr": c(Wr[sl, :].T),
        "Wo16": c(Wo[:, sl].T).astype(np.float16),
        "ub": c(u[0, hh * HC : (hh + 1) * HC, 0, :].reshape(4, P).T),
        "vb": c(v_bias[0, hh * HC : (hh + 1) * HC, 0, :].reshape(4, P).T),
        "atril": np.triu(np.ones((P, P), np.uint8), k=1),
    }


def combine_outputs(results):
    # results: list of 8 dicts with "out" [S, D]; partial sums per batch pair
    out = np.empty((4, S, D), np.float32)
    for b in range(4):
        out[b] = results[2 * b]["out"] + results[2 * b + 1]["out"]
    return out


_CACHED_NC = None
last_result = None  # BassKernelResults of the most recent run (for test harness)


def kernel(q, k, v, mask, u, v_bias, Wq, Wk, Wv, Wr, Wo, R):
    global _CACHED_NC, last_result
    from concourse.bass_utils import run_bass_kernel_spmd

    q, k, v = np.asarray(q), np.asarray(k), np.asarray(v)
    u, v_bias = np.asarray(u), np.asarray(v_bias)
    Wq, Wk, Wv, Wr, Wo, R = map(np.asarray, (Wq, Wk, Wv, Wr, Wo, R))

    # The kernel exploits the known TXL mask structure (j <= i + MEM).
    # Verify the passed mask matches; structural masking is baked in.
    m = np.asarray(mask)
    exp_mask = (np.arange(T)[None, :] <= np.arange(S)[:, None] + 1024)
    assert m.shape == (4, S, T) and bool((m == exp_mask[None]).all()), \
        "kernel compiled for the TXL causal mask (j <= i + MEM)"

    if _CACHED_NC is None:
        _CACHED_NC = build_nc()

    in_maps = [prep_core_inputs(c, q, k, v, u, v_bias, Wq, Wk, Wv, Wr, Wo, R)
               for c in range(8)]
    trace = bool(os.environ.get("TXL_TRACE"))
    last_result = run_bass_kernel_spmd(
        _CACHED_NC, in_maps, list(range(8)), trace=trace,
        trace_cores=[0] if trace else None)
    return combine_outputs(last_result.results)



# revision 8
# speedup vs baseline: 2.7421x; 2.7421x over previous
"""Transformer-XL relative multi-head attention, 8-way sharded on Trainium2.

Self-contained harness entry: kernel(**inputs) -> np.ndarray [4, 1024, 1024].

Sharding: core c handles batch b = c//2 and head-half hh = c%2 (8 of 16
heads). Each core computes a partial output (its heads' contribution
through Wo); the host unshard sums the two partials per batch (row-parallel
tensor parallelism for the output projection).

Kernel structure ("transposed-scores" form):
  - projections q/k/v/R (bf16 inputs, batched DMA loads)
  - position matrix M = (qh+v)^T . rh2 per query tile, exp'd on the scalar
    engine, written to a DRAM scratch region with row stride W; the
    circulant shift is realized by reading back with row stride W+1
    through a single DMA-transpose per (quad, key-block), which lands
    exp(bd)^T [key, query] tiles directly in SBUF.
  - content scores computed transposed (ac^T = kh^T-block . qu) so the
    AV matmul needs no attention transpose at all; exp on scalar engine.
  - att = exp(ac)*exp(bd) (one fp16 vector multiply); AV accumulates over
    key blocks with an appended ones-column in vh to produce the softmax
    denominator; normalization applied after AV via a partition-broadcast
    reciprocal.
  - heads h0/h1 of each pair are issued back-to-back so their K=64
    matmuls pack into the two row-halves of the PE array.
"""

import os
import sys

sys.path.insert(0, "/opt/trn_rl_repo")

import numpy as np
import ml_dtypes

import concourse.bass as bass
import concourse.mybir as mybir
from concourse.tile import TileContext, ScopedClock

F32 = mybir.dt.float32
F16 = mybir.dt.float16
BF16 = mybir.dt.bfloat16
AF = mybir.ActivationFunctionType
OP = mybir.AluOpType

S, T, D, DK, P = 1024, 2048, 1024, 64, 128
DH = 512          # head-slice width per core (8 heads)
HC = 8
BIAS_H = -3.5     # exp bias applied to each of the two exp factors
W_QUAD = (1663, 2175)              # shear width per quad (= w_of(quad max qi))
SQ = (128 * 1664, 128 * 2176)      # DRAM slot stride per qi within a quad
QBASE = (0, 4 * SQ[0])             # quad block offsets inside a (hp,h) region
MREG = 4 * SQ[0] + 4 * SQ[1]       # elems per (hp, h) scratch region


def kq_of(qi):  # valid key count for query tile qi (keys j <= i + 1024)
    return (qi + 9) * P


def w_of(qi):  # position-matrix width for query tile qi
    return kq_of(qi) + 127


def cdiv(a, b):
    return (a + b - 1) // b


def _patched_drain_and_barrier(self, tick_clock, wait_clock):
    # The walrus build in this container caps sync-waits per instruction;
    # Tile's stock tail drain carries one wait per live proc. Emit one SP nop
    # per wait instead, then the drain.
    dummy = mybir.InstNoOp(name="drain-wait-probe", ins=[], outs=[])
    dummy.engine = mybir.EngineType.SP
    wait_clock.add_sem_waits(dummy, ScopedClock({None: tick_clock.global_clock}))
    waits = []
    if dummy.sync_info is not None and dummy.sync_info.on_wait:
        waits = [(w.ant_name, w.wait_value) for w in dummy.sync_info.on_wait]
    assert self.sems is not None
    name2sem = {h.name: h for h in self.sems.allocated().values()}
    for name, val in waits:
        self.nc.sync.nop().wait_op(name2sem[name], val, "sem-ge")
    self.nc.sync.drain()
    self.nc.all_engine_barrier()
    popped = self.nc._tile_sem_poison_stack.pop()
    assert popped is self._sem_poison
    self.nc.clear_and_free_semaphores(list(self.sems.allocated().values()))
    self.nc.all_engine_barrier()


TileContext._drain_and_barrier = _patched_drain_and_barrier


def _split_multi_waits(nc, max_waits=1):
    """Walrus in this container rejects instructions carrying more than a
    couple of sync waits. Hoist extras onto same-engine NoOps just before
    the instruction (sequential on the engine, so semantics unchanged)."""
    for f in nc.m.functions:
        for bb in f.blocks:
            out = []
            changed = False
            for inst in bb.instructions:
                si = inst.sync_info
                if si is not None and si.on_wait and len(si.on_wait) > max_waits:
                    waits = list(si.on_wait)
                    for j, w in enumerate(waits[:-max_waits]):
                        nop = mybir.InstNoOp(
                            name=f"{inst.name}-wsplit{j}", ins=[], outs=[])
                        nop.engine = inst.engine
                        nop.sync_info = mybir.SyncInfo(on_wait=[w], on_update=[])
                        out.append(nop)
                    inst.sync_info = mybir.SyncInfo(
                        on_wait=waits[-max_waits:],
                        on_update=list(si.on_update))
                    changed = True
                out.append(inst)
            if changed:
                bb.instructions = out


def build_nc(split_waits=True):
    nc = bass.Bass(target_bir_lowering=True)

    qT = nc.declare_dram_parameter("qT", [D, S], BF16, isOutput=False)
    kT = nc.declare_dram_parameter("kT", [D, T], BF16, isOutput=False)
    vT = nc.declare_dram_parameter("vT", [D, T], BF16, isOutput=False)
    RT = nc.declare_dram_parameter("RT", [D, T], BF16, isOutput=False)
    Wq = nc.declare_dram_parameter("Wq", [D, DH], BF16, isOutput=False)
    Wk = nc.declare_dram_parameter("Wk", [D, DH], BF16, isOutput=False)
    Wv = nc.declare_dram_parameter("Wv", [D, DH], BF16, isOutput=False)
    Wr = nc.declare_dram_parameter("Wr", [D, DH], BF16, isOutput=False)
    Wo16 = nc.declare_dram_parameter("Wo16", [DH, D], F16, isOutput=False)
    ub = nc.declare_dram_parameter("ub", [P, 4], F32, isOutput=False)
    vb = nc.declare_dram_parameter("vb", [P, 4], F32, isOutput=False)
    atril2 = nc.declare_dram_parameter(
        "atril2", [P, P], mybir.dt.uint8, isOutput=False)
    outp = nc.declare_dram_parameter("out", [S, D], F32, isOutput=True)

    with TileContext(nc) as tc:
        with (
            tc.tile_pool(name="persist", bufs=1) as pp,
            tc.tile_pool(name="consts", bufs=1) as cp,
        ):
            # persistent fp16 tensors (partition = dk of the 2 heads in a pair)
            quT = pp.tile([P, 4 * S], F16)       # (qh+u).T   blocks hp
            qvT = pp.tile([P, 4 * S], F16)       # (qh+v).T
            khT = pp.tile([P, 4 * T], F16)
            rh2T = pp.tile([P, 4 * 3072], F16)
            vh16 = pp.tile([P, 16 * (HC * 65)], F16)  # per key tile: 8 h x (64+1)
            concatT = pp.tile([P, 4 * S], F16)
            WoS = pp.tile([P, 4 * D], F16)

            ub_sb = cp.tile([P, 4], F32)
            vb_sb = cp.tile([P, 4], F32)
            atril2_sb = cp.tile([P, P], mybir.dt.uint8)
            zeros_sb = cp.tile([P, P], F16)
            biasn_sb = cp.tile([P, 1], F32)
            ones_sb = cp.tile([P, DK], F32)
            nc.vector.memset(biasn_sb[:], BIAS_H)
            nc.vector.memset(zeros_sb[:], 0.0)
            nc.vector.memset(ones_sb[:], 1.0)

            nc.scalar.dma_start(out=ub_sb[:], in_=ub[:])
            nc.scalar.dma_start(out=vb_sb[:], in_=vb[:])
            nc.scalar.dma_start(out=atril2_sb[:], in_=atril2[:])
            # WoS layout [128, dt*1024 + o] <- Wo16[(dt p), o], one DMA
            nc.scalar.dma_start(
                out=WoS[:],
                in_=bass.AP(Wo16, 0, [[D, P], [P * D, 4], [1, D]]),
            )

            # ---------------- projections ----------------
            def load_w(pool, wparam):
                wsb = pool.tile([P, 8 * DH], BF16, tag="wsb")
                nc.scalar.dma_start(
                    out=wsb[:],
                    in_=bass.AP(wparam, 0, [[DH, P], [P * DH, 8], [1, DH]]),
                )
                return wsb

            def load_x(pool, xparam, th):
                # [128, 8*1024]: all 8 contraction chunks of one 1024-col slab
                xsb = pool.tile([P, 8 * 1024], BF16, tag="xstage")
                L = xparam.shape[1]
                nc.scalar.dma_start(
                    out=xsb[:],
                    in_=bass.AP(xparam, th * 1024, [[L, P], [P * L, 8], [1, 1024]]),
                )
                return xsb

            def proj_T(pool, psum, wsb, xparam, ncols, evac):
                nth = ncols // 1024
                for th in range(nth):
                    xsb = load_x(pool, xparam, th)
                    psums = {k: psum.tile([P, 512], F32, tag="proj", name="projps")
                             for k in [(d, t2) for d in range(4) for t2 in range(2)]}
                    for kd in range(8):
                        for dot in range(4):
                            for tc2 in range(2):
                                nc.tensor.matmul(
                                    psums[(dot, tc2)][:],
                                    wsb[:, kd * DH + dot * P : kd * DH + (dot + 1) * P],
                                    xsb[:, kd * 1024 + tc2 * 512 : kd * 1024 + (tc2 + 1) * 512],
                                    start=(kd == 0),
                                    stop=(kd == 7),
                                )
                    for dot in range(4):
                        for tc2 in range(2):
                            evac(psums[(dot, tc2)], dot, th * 1024 + tc2 * 512)

            with (
                tc.tile_pool(name="projp", bufs=2) as jp,
                tc.tile_pool(name="projw", bufs=2) as jw,
                tc.tile_pool(name="rhtmp", bufs=1) as jr,
                tc.tile_pool(name="projpsum", bufs=8, space="PSUM") as jps,
            ):
                wsb = load_w(jw, Wq)

                def evac_q(ps, dot, col):
                    nc.vector.tensor_scalar(
                        quT[:, dot * S + col : dot * S + col + 512], ps[:],
                        ub_sb[:, dot : dot + 1], None, OP.add)
                    nc.vector.tensor_scalar(
                        qvT[:, dot * S + col : dot * S + col + 512], ps[:],
                        vb_sb[:, dot : dot + 1], None, OP.add)

                proj_T(jp, jps, wsb, qT, S, evac_q)

                wsb = load_w(jw, Wk)

                def evac_k(ps, dot, col):
                    nc.vector.tensor_copy(
                        khT[:, dot * T + col : dot * T + col + 512], ps[:])

                proj_T(jp, jps, wsb, kT, T, evac_k)

                rhT = jr.tile([P, 4 * T], F16, tag="rhT")
                wsb = load_w(jw, Wr)

                def evac_r(ps, dot, col):
                    nc.vector.tensor_copy(
                        rhT[:, dot * T + col : dot * T + col + 512], ps[:])

                proj_T(jp, jps, wsb, RT, T, evac_r)

                # rh2T[:, m'] = rhT[:, (m' + 1023) % 2048], m' in [0, 3072)
                for dot in range(4):
                    nc.vector.tensor_copy(
                        rh2T[:, dot * 3072 : dot * 3072 + 1025],
                        rhT[:, dot * T + 1023 : dot * T + 2048])
                    nc.vector.tensor_copy(
                        rh2T[:, dot * 3072 + 1025 : dot * 3072 + 3072],
                        rhT[:, dot * T : dot * T + 2047])

                # vh (untransposed): per key tile tt, psum [128 keys, 512 dh]
                wsb = load_w(jw, Wv)
                for tg in range(2):
                    vsb = load_x(jp, vT, tg)
                    vps = {tl: jps.tile([P, 512], F32, tag="proj", name="vhps")
                           for tl in range(8)}
                    for kd in range(8):
                        for tl in range(8):
                            nc.tensor.matmul(
                                vps[tl][:],
                                vsb[:, kd * 1024 + tl * P : kd * 1024 + (tl + 1) * P],
                                wsb[:, kd * DH : (kd + 1) * DH],
                                start=(kd == 0),
                                stop=(kd == 7),
                            )
                    for tl in range(8):
                        tt = tg * 8 + tl
                        ps = vps[tl]
                        base = tt * (HC * 65)
                        dst = bass.AP(vh16.tensor, vh16.offset + base,
                                      [[vh16.tensor.shape[1], P], [65, HC], [1, DK]])
                        nc.vector.tensor_copy(
                            dst, ps[:].rearrange("p (h c) -> p h c", h=HC))
                        ones = bass.AP(vh16.tensor, vh16.offset + base + DK,
                                       [[vh16.tensor.shape[1], P], [65, HC]])
                        nc.vector.memset(ones, 1.0)

            # ---------------- attention ----------------
            with (
                tc.tile_pool(name="att_m", bufs=4) as mp,
                tc.tile_pool(name="att_eac", bufs=12) as ep,
                tc.tile_pool(name="att_ebd", bufs=6) as bp,
                tc.tile_pool(name="att_att", bufs=6) as atp,
                tc.tile_pool(name="dram", bufs=3, space="DRAM") as dp,
                tc.tile_pool(name="nrm", bufs=2) as np_,
                tc.tile_pool(name="nrmb", bufs=4) as npb,
                tc.tile_pool(name="ps_m", bufs=2, space="PSUM") as psm,
                tc.tile_pool(name="ps_ac", bufs=2, space="PSUM") as psac,
                tc.tile_pool(name="ps_av", bufs=4, space="PSUM") as psav,
            ):
                for hp in range(4):
                    mreg = [dp.tile([MREG], F16, tag="mreg", name="mreg") for _ in range(2)]

                    # ---- M stage: position matrix, exp'd, shear-written ----
                    for qi in range(8):
                        Qd = qi // 4
                        Wqd = W_QUAD[Qd]
                        Wq_ = w_of(qi)
                        nwc = cdiv(Wq_, 512)
                        msbs = [mp.tile([P, 2176], F16, tag="msb", name="msb")
                                for _ in range(2)]
                        for wc in range(nwc):
                            nw = min(512, Wq_ - wc * 512)
                            for h in range(2):
                                pr = slice(h * DK, (h + 1) * DK)
                                mps = psm.tile([P, 512], F32, tag="mps")
                                nc.tensor.matmul(
                                    mps[:, :nw],
                                    qvT[pr, hp * S + qi * P : hp * S + (qi + 1) * P],
                                    rh2T[pr, hp * 3072 + qi * P + wc * 512 :
                                         hp * 3072 + qi * P + wc * 512 + nw],
                                    start=True, stop=True,
                                )
                                nc.scalar.activation(
                                    msbs[h][:, wc * 512 : wc * 512 + nw],
                                    mps[:, :nw], AF.Exp,
                                    bias=biasn_sb[:], scale=0.125)
                        base = QBASE[Qd] + (qi % 4) * SQ[Qd]
                        for h in range(2):
                            nc.sync.dma_start(
                                out=bass.AP(mreg[h].tensor,
                                            mreg[h].offset + base,
                                            [[Wqd, P], [1, Wq_]]),
                                in_=msbs[h][:, :Wq_],
                            )

                    # ---- quad stage: ac^T, combine, AV ----
                    avps_all = {}
                    for Q in range(2):
                        avps = [psav.tile([P, 512], F32, tag="avps", name="avps")
                                for _ in range(2)]
                        for h in range(2):
                            avps_all[(Q, h)] = avps[h]
                        nkb = 12 if Q == 0 else 16
                        eacs = {}

                        def emit_ac(kb, Q=Q, eacs=eacs):
                            for h in range(2):
                                pr = slice(h * DK, (h + 1) * DK)
                                qs = max(4 * Q, kb - 8)
                                n_q = (4 * Q + 4 - qs) * P
                                acps = psac.tile([P, 512], F32, tag="acps")
                                nc.tensor.matmul(
                                    acps[:, :n_q],
                                    khT[pr, hp * T + kb * P : hp * T + (kb + 1) * P],
                                    quT[pr, hp * S + qs * P : hp * S + qs * P + n_q],
                                    start=True, stop=True,
                                )
                                eac = ep.tile([P, 512], F16, tag="eac")
                                nc.scalar.activation(
                                    eac[:, :n_q], acps[:, :n_q], AF.Exp,
                                    bias=biasn_sb[:], scale=0.125)
                                if kb >= 4 * Q + 8:
                                    # causal boundary block: keys jl > il invalid
                                    nc.vector.copy_predicated(
                                        eac[:, 0:P], atril2_sb[:], zeros_sb[:])
                                eacs[(h, kb)] = (eac, qs, n_q)

                        def emit_av(kb, last, Q=Q, eacs=eacs, avps=avps,
                                    mreg=mreg):
                            Wqd = W_QUAD[Q]
                            for h in range(2):
                                eac, qs, n_q = eacs.pop((h, kb))
                                ebd = bp.tile([P, 512], F16, tag="ebd")
                                src = bass.AP(
                                    mreg[h].tensor,
                                    mreg[h].offset + QBASE[Q]
                                    + (qs - 4 * Q) * SQ[Q] + kb * P,
                                    [[Wqd + 1, n_q], [1, P]],
                                )
                                nc.sync.dma_start_transpose(
                                    out=ebd[:, :n_q], in_=src)
                                att = atp.tile([P, 512], F16, tag="att")
                                nc.vector.tensor_tensor(
                                    att[:, :n_q], eac[:, :n_q], ebd[:, :n_q],
                                    OP.mult)
                                qloc = (qs - 4 * Q) * P
                                col = kb * (HC * 65) + (hp * 2 + h) * 65
                                nc.tensor.matmul(
                                    avps[h][0:65, qloc : qloc + n_q],
                                    vh16[:, col : col + 65],
                                    att[:, :n_q],
                                    start=(kb == 0), stop=last,
                                    skip_group_check=True,
                                )

                        LOOK = 5
                        for i in range(min(LOOK, nkb)):
                            emit_ac(i)
                        for i in range(nkb):
                            if i + LOOK < nkb:
                                emit_ac(i + LOOK)
                            emit_av(i, last=(i == nkb - 1))

                    # ---- normalization for this hp (batched reciprocal) ----
                    # engine partition bases must be 32-aligned: park the four
                    # denominator rows on partitions 0/32/64/96
                    dcol = np_.tile([P, 512], F32, tag="dcol")
                    rcol = np_.tile([P, 512], F32, tag="rcol")
                    nc.vector.memset(dcol[:], 1.0)
                    for Q in range(2):
                        for h in range(2):
                            c = 32 * (Q * 2 + h)
                            nc.scalar.activation(
                                dcol[c : c + 1, :],
                                avps_all[(Q, h)][64:65, :], AF.Copy)
                    nc.vector.reciprocal(rcol[:], dcol[:])
                    for Q in range(2):
                        for h in range(2):
                            c = 32 * (Q * 2 + h)
                            # broadcast recip row across 64 partitions via a
                            # K=1 matmul: rbc = ones[1,64]^T @ recip[1,512]
                            rbp = psac.tile([P, 512], F32, tag="acps",
                                            name="rbp")
                            nc.tensor.matmul(
                                rbp[0:DK, :], ones_sb[c : c + 1, :],
                                rcol[c : c + 1, :], start=True, stop=True,
                                tile_position=(c, 0))
                            rbc = npb.tile([DK, 512], F32, tag="rbc")
                            nc.vector.tensor_copy(rbc[:], rbp[0:DK, :])
                            nc.vector.tensor_tensor(
                                concatT[h * DK : (h + 1) * DK,
                                        hp * S + Q * 512 : hp * S + (Q + 1) * 512],
                                avps_all[(Q, h)][0:DK, :], rbc[:], OP.mult)

            # ---------------- output projection ----------------
            with (
                tc.tile_pool(name="outp", bufs=2) as op_,
                tc.tile_pool(name="outpsum", bufs=4, space="PSUM") as ops_,
            ):
                for it in range(8):
                    osb = op_.tile([P, 1024], F32, tag="osb")
                    for oc in range(2):
                        ps = ops_.tile([P, 512], F32, tag="out")
                        for dt in range(4):
                            nc.tensor.matmul(
                                ps[:],
                                concatT[:, dt * S + it * P : dt * S + (it + 1) * P],
                                WoS[:, dt * D + oc * 512 : dt * D + (oc + 1) * 512],
                                start=(dt == 0), stop=(dt == 3),
                            )
                        nc.vector.tensor_copy(osb[:, oc * 512 : (oc + 1) * 512], ps[:])
                    nc.sync.dma_start(
                        out=outp[it * P : (it + 1) * P, :], in_=osb[:])

    if split_waits:
        _split_multi_waits(nc)
    return nc


def prep_core_inputs(core, q, k, v, u, v_bias, Wq, Wk, Wv, Wr, Wo, R):
    b, hh = core // 2, core % 2
    sl = slice(hh * DH, (hh + 1) * DH)
    BF = ml_dtypes.bfloat16
    return {
        "qT": q[b].T.astype(BF),
        "kT": k[b].T.astype(BF),
        "vT": v[b].T.astype(BF),
        "RT": R.T.astype(BF),
        "Wq": Wq[sl, :].T.astype(BF),
        "Wk": Wk[sl, :].T.astype(BF),
        "Wv": Wv[sl, :].T.astype(BF),
        "Wr": Wr[sl, :].T.astype(BF),
        "Wo16": Wo[:, sl].T.astype(np.float16),
        "ub": np.ascontiguousarray(
            u[0, hh * HC : (hh + 1) * HC, 0, :].reshape(4, P).T),
        "vb": np.ascontiguousarray(
            v_bias[0, hh * HC : (hh + 1) * HC, 0, :].reshape(4, P).T),
        "atril2": np.tril(np.ones((P, P), np.uint8), -1),
    }


def combine_outputs(results):
    # results: list of 8 dicts with "out" [S, D]; partial sums per batch pair
    out = np.empty((4, S, D), np.float32)
    for b in range(4):
        out[b] = results[2 * b]["out"] + results[2 * b + 1]["out"]
    return out


_CACHED_NC = None
last_result = None  # BassKernelResults of the most recent run (for test harness)


def kernel(q, k, v, mask, u, v_bias, Wq, Wk, Wv, Wr, Wo, R):
    global _CACHED_NC, last_result
    from concourse.bass_utils import run_bass_kernel_spmd

    q, k, v = np.asarray(q), np.asarray(k), np.asarray(v)
    u, v_bias = np.asarray(u), np.asarray(v_bias)
    Wq, Wk, Wv, Wr, Wo, R = map(np.asarray, (Wq, Wk, Wv, Wr, Wo, R))

    # The kernel exploits the known TXL mask structure (j <= i + MEM).
    # Verify the passed mask matches; structural masking is baked in.
    m = np.asarray(mask)
    exp_mask = (np.arange(T)[None, :] <= np.arange(S)[:, None] + 1024)
    assert m.shape == (4, S, T) and bool((m == exp_mask[None]).all()), \
        "kernel compiled for the TXL causal mask (j <= i + MEM)"

    if _CACHED_NC is None:
        _CACHED_NC = build_nc()

    in_maps = [prep_core_inputs(c, q, k, v, u, v_bias, Wq, Wk, Wv, Wr, Wo, R)
               for c in range(8)]
    trace = bool(os.environ.get("TXL_TRACE"))
    last_result = run_bass_kernel_spmd(
        _CACHED_NC, in_maps, list(range(8)), trace=trace,
        trace_cores=[0] if trace else None)
    return combine_outputs(last_result.results)
